# revision 15
# baseline (speedup 1.0000x reference)
"""MoE transformer block (attention + top-2 MoE FFN) on 8 Trainium2 cores.

Sharding: token-parallel. Core c handles batch c//4, query chunk (c%4)*512.
Each core receives its batch's tokens ROLLED so that its query chunk sits at
rows 0..511 — the compiled program is identical across cores (pure SPMD) and
all per-core variation lives in the input data (x, rope tables, mask columns).

Host-side folding: norm1_w into q/k/v weights, norm2_w into router/gate_up,
q/k-norm weights and the 1/sqrt(HD) score scale into the rope cos/sin tables.
Matmuls run in bf16 with f32 PSUM accumulation; softmax and rmsnorm run in
f32; the router path (h2 -> logits) stays f32 so top-2 expert selection
matches the f32 reference.  MoE is computed densely (all 8 experts) as two
stacked matmuls; the top-2 combine weights are zero for unselected experts
and are folded into the activation in expert-major layout.  All bf16
activation transposes go through the DMA xbar (dma_start_transpose), keeping
PE/DVE free for matmuls and evictions.
"""

import sys
from contextlib import ExitStack

sys.path.insert(0, "/opt/trn_rl_repo")

import numpy as np
import ml_dtypes

import concourse.bass as bass
import concourse.mybir as mybir
import concourse.tile as tile
from concourse.vector_clock import ScopedClock
from concourse.masks import make_identity
from concourse.bass_utils import run_bass_kernel_spmd

# ---------------------------------------------------------------- constants
B, S, EMB = 2, 2048, 1024
NH, NKV, HD = 16, 4, 128
NE, MH = 8, 1024
CH = 512  # query tokens per core
P = 128
NT = S // P  # 16 token tiles
NQ = CH // P  # 4 query tiles
EPS = 1e-6
ROPE_BASE = 10000.0

F32 = mybir.dt.float32
BF16 = mybir.dt.bfloat16
AF = mybir.ActivationFunctionType
ALU = mybir.AluOpType
AX = mybir.AxisListType
NPBF = ml_dtypes.bfloat16

# ------------------------------------------------- walrus single-wait patch
_uid = [0]


class _SplitWaitTileContext(tile.TileContext):
    """This container's walrus build rejects instructions carrying more than
    one sync wait; hoist extra waits onto same-engine single-wait NoOps."""

    def _add_instruction(self, inst):
        si = inst.sync_info
        if si is not None and len(si.on_wait) > 1:
            waits = list(si.on_wait)
            for w in waits[:-1]:
                _uid[0] += 1
                nop = mybir.InstNoOp(
                    name=f"WSPLIT-{_uid[0]}",
                    engine=inst.engine,
                    ins=[],
                    outs=[],
                    sync_info=mybir.SyncInfo(on_wait=[w], on_update=[]),
                )
                super()._add_instruction(nop)
            inst.sync_info = mybir.SyncInfo(
                on_wait=[waits[-1]], on_update=list(si.on_update)
            )
        super()._add_instruction(inst)

    def _drain_and_barrier(self, tick_clock, wait_clock):
        nc = self.nc
        drain_inst = nc.sync.drain()
        wait_clock.add_sem_waits(
            drain_inst.ins, ScopedClock({None: tick_clock.global_clock})
        )
        si = drain_inst.ins.sync_info
        if si is not None and len(si.on_wait) > 1:
            waits = list(si.on_wait)
            drain_inst.ins.sync_info = mybir.SyncInfo(
                on_wait=[waits[0]], on_update=list(si.on_update)
            )
            for w in waits[1:]:
                nop = nc.sync.nop(nofuse=True)
                nop.ins.sync_info = mybir.SyncInfo(on_wait=[w], on_update=[])
        nc.all_engine_barrier()
        assert self.sems is not None
        popped = nc._tile_sem_poison_stack.pop()
        assert popped is self._sem_poison
        nc.clear_and_free_semaphores(list(self.sems.allocated().values()))
        nc.all_engine_barrier()


# ------------------------------------------------------------ program build
def _build(mask_mode: str) -> bass.Bass:
    """mask_mode: 'zero' (mask known all-zero, skip the add) or 'general'."""
    nc = bass.Bass()

    x_in = nc.declare_dram_parameter("x", [S, EMB], F32, isOutput=False)
    cosq = nc.declare_dram_parameter("cosq", [CH, HD], F32, isOutput=False)
    sinq = nc.declare_dram_parameter("sinq", [CH, HD], F32, isOutput=False)
    cosk = nc.declare_dram_parameter("cosk", [S, HD], F32, isOutput=False)
    sink = nc.declare_dram_parameter("sink", [S, HD], F32, isOutput=False)
    qwT = nc.declare_dram_parameter("qwT", [8, 4, P, 512], BF16, isOutput=False)
    kwT = nc.declare_dram_parameter("kwT", [8, P, 512], BF16, isOutput=False)
    vwT = nc.declare_dram_parameter("vwT", [8, P, 512], BF16, isOutput=False)
    owT = nc.declare_dram_parameter("owT", [16, 2, P, 512], BF16, isOutput=False)
    rwT = nc.declare_dram_parameter("rwT", [8, P, 8], F32, isOutput=False)
    w1 = nc.declare_dram_parameter("w1", [128, P, 1024], BF16, isOutput=False)
    w2 = nc.declare_dram_parameter("w2", [16, P, 4096], BF16, isOutput=False)
    if mask_mode == "general":
        mask_in = nc.declare_dram_parameter("mask", [S, CH], BF16, isOutput=False)
    y_out = nc.declare_dram_parameter("y", [CH, EMB], F32, isOutput=True)

    combT_d = nc.dram_tensor("combT_d", [NE, CH], F32)
    rcp_d = nc.dram_tensor("rcp_d", [NH, CH], F32)

    with _SplitWaitTileContext(nc) as tc:
        with ExitStack() as top:
            const = top.enter_context(tc.tile_pool(name="const", bufs=1))
            ident_f = const.tile([P, P], F32, tag="identf", name="identf")
            make_identity(nc, ident_f)
            eps_t = const.tile([P, 1], F32, tag="epst", name="epst")
            nc.vector.memset(eps_t[:], EPS)
            ones_bf = const.tile([P, 1], BF16, tag="onesbf", name="onesbf")
            nc.vector.memset(ones_bf[:], 1.0)

            # persistent across attention
            xattn_p = top.enter_context(tc.tile_pool(name="xattn", bufs=NQ))
            xattn = [xattn_p.tile([P, EMB], F32, tag="xattn", name="xattn")
                     for _ in range(NQ)]

            with ExitStack() as attn_stack:
                ctxT_p = attn_stack.enter_context(tc.tile_pool(name="ctxT", bufs=NH))
                ctxT = [ctxT_p.tile([P, CH], BF16, tag="ctxT", name="ctxT")
                        for _ in range(NH)]

                with ExitStack() as qkv_stack:
                    kvq_p = qkv_stack.enter_context(tc.tile_pool(name="kvq", bufs=1))
                    kT = kvq_p.tile([P, NKV, S], BF16, tag="kTb", name="kTb")
                    vB = kvq_p.tile([P, NT, 512], BF16, tag="vB", name="vB")
                    qT = kvq_p.tile([P, NH, CH], BF16, tag="qTb", name="qTb")

                    # ---------- phase 1: rmsnorm(x) -> xhatT (bf16 feature-major)
                    with ExitStack() as ph1:
                        xh_p = ph1.enter_context(tc.tile_pool(name="xhT", bufs=1))
                        xhatT = xh_p.tile([P, EMB // P, S], BF16, tag="xhT", name="xhT")
                        with tc.tile_pool(name="ph1s", bufs=3) as sp, \
                             tc.tile_pool(name="ph1b", bufs=3) as bp, \
                             tc.tile_pool(name="ph1ss", bufs=4) as ssp:
                            for t in range(NT):
                                xt = sp.tile([P, EMB], F32, tag="xt", name="xt")
                                nc.sync.dma_start(xt[:], x_in[t * P : (t + 1) * P, :])
                                ss = ssp.tile([P, 1], F32, tag="ss", name="ss")
                                sq1 = sp.tile([P, EMB], F32, tag="sq1", name="sq1")
                                nc.scalar.activation(
                                    sq1[:], xt[:], AF.Square, accum_out=ss[:]
                                )
                                rt = ssp.tile([P, 1], F32, tag="rt", name="rt")
                                nc.scalar.activation(
                                    rt[:], ss[:], AF.Sqrt, bias=eps_t[:], scale=1.0 / EMB
                                )
                                sc = ssp.tile([P, 1], F32, tag="sc", name="sc")
                                nc.vector.reciprocal(sc[:], rt[:])
                                xb = bp.tile([P, EMB], BF16, tag="xb", name="xb")
                                nc.vector.tensor_scalar(
                                    xb[:], xt[:], sc[:], None, op0=ALU.mult
                                )
                                nc.sync.dma_start_transpose(
                                    xhatT[:, :, t * P : (t + 1) * P], xb[:]
                                )

                        # ---------- phase 2: Q/K/V projections (+norm+rope+T)
                        with tc.tile_pool(name="tabs", bufs=NT) as tabp, \
                             tc.tile_pool(name="kwp", bufs=8) as kwp, \
                             tc.tile_pool(name="vwp", bufs=8) as vwp, \
                             tc.tile_pool(name="qwp", bufs=8) as qwp, \
                             tc.tile_pool(name="kvf", bufs=3) as kvf, \
                             tc.tile_pool(name="rope", bufs=6) as rp, \
                             tc.tile_pool(name="ropss", bufs=8) as rssp, \
                             tc.tile_pool(name="hbf", bufs=3) as hbfp, \
                             tc.tile_pool(name="kvps", bufs=4, space="PSUM") as kvps:
                            coskt = [tabp.tile([P, HD], F32, tag="coskt", name="coskt")
                                     for _ in range(NT)]
                            sinkt = [tabp.tile([P, HD], F32, tag="sinkt", name="sinkt")
                                     for _ in range(NT)]
                            cosqt = [tabp.tile([P, HD], F32, tag="cosqt", name="cosqt")
                                     for _ in range(NQ)]
                            sinqt = [tabp.tile([P, HD], F32, tag="sinqt", name="sinqt")
                                     for _ in range(NQ)]
                            for t in range(NT):
                                nc.sync.dma_start(coskt[t][:], cosk[t * P : (t + 1) * P, :])
                                nc.sync.dma_start(sinkt[t][:], sink[t * P : (t + 1) * P, :])
                            for m in range(NQ):
                                nc.sync.dma_start(cosqt[m][:], cosq[m * P : (m + 1) * P, :])
                                nc.sync.dma_start(sinqt[m][:], sinq[m * P : (m + 1) * P, :])

                            kw_sb = [kwp.tile([P, 512], BF16, tag="kw", name="kw")
                                     for _ in range(8)]
                            vw_sb = [vwp.tile([P, 512], BF16, tag="vw", name="vw")
                                     for _ in range(8)]
                            for k in range(8):
                                nc.sync.dma_start(kw_sb[k][:], kwT[k])
                                nc.sync.dma_start(vw_sb[k][:], vwT[k])

                            def norm_rope(src, cost, sint, dst):
                                """src [P,HD] f32 -> rmsnorm+rope -> bf16 into dst."""
                                ssq = rssp.tile([P, 1], F32, tag="ssq", name="ssq")
                                sqr = rp.tile([P, HD], F32, tag="sqr", name="sqr")
                                nc.scalar.activation(
                                    sqr[:], src, AF.Square, accum_out=ssq[:]
                                )
                                rtq = rssp.tile([P, 1], F32, tag="rtq", name="rtq")
                                nc.scalar.activation(
                                    rtq[:], ssq[:], AF.Sqrt, bias=eps_t[:], scale=1.0 / HD
                                )
                                scq = rssp.tile([P, 1], F32, tag="scq", name="scq")
                                nc.vector.reciprocal(scq[:], rtq[:])
                                tcos = rp.tile([P, HD], F32, tag="tcos", name="tcos")
                                nc.vector.tensor_tensor(tcos[:], src, cost[:], op=ALU.mult)
                                tsin = rp.tile([P, HD], F32, tag="tsin", name="tsin")
                                h = HD // 2
                                nc.vector.tensor_tensor(
                                    tsin[:, :h], src[:, h:], sint[:, :h], op=ALU.mult
                                )
                                nc.vector.tensor_tensor(
                                    tsin[:, h:], src[:, :h], sint[:, h:], op=ALU.mult
                                )
                                t1 = rp.tile([P, HD], F32, tag="t1", name="t1")
                                nc.vector.tensor_scalar(
                                    t1[:], tcos[:], scq[:], None, op0=ALU.mult
                                )
                                nc.vector.scalar_tensor_tensor(
                                    dst, tsin[:], scq[:], t1[:],
                                    op0=ALU.mult, op1=ALU.add,
                                )

                            # K and V over all token tiles
                            for t in range(NT):
                                ps_k = kvps.tile([P, 512], F32, tag="ps2", name="psk")
                                ps_v = kvps.tile([P, 512], F32, tag="ps2", name="psv")
                                for k in range(8):
                                    nc.tensor.matmul(
                                        ps_k[:],
                                        xhatT[:, k, t * P : (t + 1) * P],
                                        kw_sb[k][:],
                                        start=(k == 0), stop=(k == 7),
                                    )
                                for k in range(8):
                                    nc.tensor.matmul(
                                        ps_v[:],
                                        xhatT[:, k, t * P : (t + 1) * P],
                                        vw_sb[k][:],
                                        start=(k == 0), stop=(k == 7),
                                    )
                                kf = kvf.tile([P, 512], F32, tag="kf", name="kf")
                                nc.vector.tensor_copy(kf[:], ps_k[:])
                                khat = hbfp.tile([P, 512], BF16, tag="khat", name="khat")
                                for kv in range(NKV):
                                    norm_rope(
                                        kf[:, kv * HD : (kv + 1) * HD],
                                        coskt[t], sinkt[t],
                                        khat[:, kv * HD : (kv + 1) * HD],
                                    )
                                nc.sync.dma_start_transpose(
                                    kT[:, :, t * P : (t + 1) * P], khat[:]
                                )
                                nc.vector.tensor_copy(vB[:, t, :], ps_v[:])

                            # Q over the query chunk
                            for hg in range(4):
                                qw_sb = [qwp.tile([P, 512], BF16, tag="qw", name="qw")
                                         for _ in range(8)]
                                for k in range(8):
                                    nc.sync.dma_start(qw_sb[k][:], qwT[k, hg])
                                for m in range(NQ):
                                    ps_q = kvps.tile([P, 512], F32, tag="ps2", name="psq")
                                    for k in range(8):
                                        nc.tensor.matmul(
                                            ps_q[:],
                                            xhatT[:, k, m * P : (m + 1) * P],
                                            qw_sb[k][:],
                                            start=(k == 0), stop=(k == 7),
                                        )
                                    qf = kvf.tile([P, 512], F32, tag="qf", name="qf")
                                    nc.vector.tensor_copy(qf[:], ps_q[:])
                                    qhat = hbfp.tile([P, 512], BF16, tag="qhat", name="qhat")
                                    for hh in range(4):
                                        norm_rope(
                                            qf[:, hh * HD : (hh + 1) * HD],
                                            cosqt[m], sinqt[m],
                                            qhat[:, hh * HD : (hh + 1) * HD],
                                        )
                                    nc.sync.dma_start_transpose(
                                        qT[:, hg * 4 : (hg + 1) * 4, m * P : (m + 1) * P],
                                        qhat[:],
                                    )
                    # xhatT freed here

                    # ---------- phase 3: attention per head (k-major scores,
                    # exp gives attn^T directly; rowsums via ones-matmul)
                    with ExitStack() as ph3:
                        if mask_mode == "general":
                            mk_p = ph3.enter_context(tc.tile_pool(name="mask", bufs=NT))
                            mkT = [mk_p.tile([P, CH], BF16, tag="mkT", name="mkT")
                                   for _ in range(NT)]
                            for kt in range(NT):
                                nc.sync.dma_start(
                                    mkT[kt][:], mask_in[kt * P : (kt + 1) * P, :]
                                )
                        attnT_p = ph3.enter_context(tc.tile_pool(name="attnT", bufs=2))
                        sc_p = ph3.enter_context(tc.tile_pool(name="scf", bufs=4))
                        rr_p = ph3.enter_context(tc.tile_pool(name="rr", bufs=4))
                        rep_p = ph3.enter_context(tc.tile_pool(name="rep", bufs=2))
                        ps_s = ph3.enter_context(
                            tc.tile_pool(name="pss", bufs=3, space="PSUM"))
                        ps_c = ph3.enter_context(
                            tc.tile_pool(name="psc", bufs=2, space="PSUM"))
                        ps_r = ph3.enter_context(
                            tc.tile_pool(name="psr3", bufs=2, space="PSUM"))

                        for h in range(NH):
                            kv = h // (NH // NKV)
                            attnT = attnT_p.tile([P, NT, CH], BF16, tag="attnT",
                                                 name="attnT")
                            ps_sum = ps_r.tile([1, CH], F32, tag="psum3", name="psum3")
                            for kt in range(NT):
                                pss = ps_s.tile([P, CH], F32, tag="pss", name="pss")
                                nc.tensor.matmul(
                                    pss[:],
                                    kT[:, kv, kt * P : (kt + 1) * P],
                                    qT[:, h, :],
                                    start=True, stop=True,
                                )
                                if mask_mode == "general":
                                    scf = sc_p.tile([P, CH], F32, tag="scf", name="scf")
                                    nc.vector.tensor_tensor(
                                        scf[:], pss[:], mkT[kt][:], op=ALU.add
                                    )
                                    src3 = scf
                                else:
                                    src3 = pss
                                nc.scalar.activation(
                                    attnT[:, kt, :], src3[:], AF.Exp
                                )
                                nc.tensor.matmul(
                                    ps_sum[:], ones_bf[:], attnT[:, kt, :],
                                    start=(kt == 0), stop=(kt == NT - 1),
                                )
                            rcp_row = rr_p.tile([1, CH], F32, tag="rcpr", name="rcpr")
                            nc.vector.reciprocal(rcp_row[:], ps_sum[:])
                            nc.sync.dma_start(rcp_d[h : h + 1, :], rcp_row[:])
                            rcp_rep = rep_p.tile([P, CH], F32, tag="rcprep",
                                                 name="rcprep")
                            nc.sync.dma_start(
                                rcp_rep[:], rcp_d[h : h + 1, :].partition_broadcast(P)
                            )
                            psc = ps_c.tile([P, CH], F32, tag="psc", name="psc")
                            for kt in range(NT):
                                nc.tensor.matmul(
                                    psc[:],
                                    vB[:, kt, kv * P : (kv + 1) * P],
                                    attnT[:, kt, :],
                                    start=(kt == 0), stop=(kt == NT - 1),
                                )
                            nc.vector.tensor_tensor(
                                ctxT[h][:], psc[:], rcp_rep[:], op=ALU.mult
                            )
                # kT / vB / qT freed here

                # ---------- phase 4: o_proj + residual
                with tc.tile_pool(name="ow", bufs=16) as owp, \
                     tc.tile_pool(name="xq", bufs=NQ) as xqp, \
                     tc.tile_pool(name="pso", bufs=3, space="PSUM") as pso:
                    xq = [xqp.tile([P, EMB], F32, tag="xq", name="xq")
                          for _ in range(NQ)]
                    for m in range(NQ):
                        nc.sync.dma_start(xq[m][:], x_in[m * P : (m + 1) * P, :])
                    for n in range(2):
                        ow_sb = [owp.tile([P, 512], BF16, tag="ow", name="ow")
                                 for _ in range(16)]
                        for k in range(16):
                            nc.sync.dma_start(ow_sb[k][:], owT[k, n])
                        for m in range(NQ):
                            ps = pso.tile([P, 512], F32, tag="pso", name="pso")
                            for k in range(16):
                                nc.tensor.matmul(
                                    ps[:],
                                    ctxT[k][:, m * P : (m + 1) * P],
                                    ow_sb[k][:],
                                    start=(k == 0), stop=(k == 15),
                                )
                            nc.vector.tensor_tensor(
                                xattn[m][:, n * 512 : (n + 1) * 512],
                                ps[:], xq[m][:, n * 512 : (n + 1) * 512],
                                op=ALU.add,
                            )
            # ctxT freed here

            # ---------- phase 5: h2, router, top-2 comb
            h2bf_p = top.enter_context(tc.tile_pool(name="h2bf", bufs=1))
            h2bf = h2bf_p.tile([P, EMB // P, CH], BF16, tag="h2bf", name="h2bf")
            crep_p = top.enter_context(tc.tile_pool(name="crep", bufs=NE))
            crep = [crep_p.tile([P, CH], F32, tag="crep", name="crep")
                    for _ in range(NE)]

            with tc.tile_pool(name="h2f", bufs=EMB // P) as h2fp, \
                 tc.tile_pool(name="rw", bufs=8) as rwp, \
                 tc.tile_pool(name="r5s", bufs=8) as r5s, \
                 tc.tile_pool(name="r5b", bufs=3) as r5b, \
                 tc.tile_pool(name="combT", bufs=1) as combp, \
                 tc.tile_pool(name="ps5", bufs=2, space="PSUM") as ps5, \
                 tc.tile_pool(name="ps5t", bufs=2, space="PSUM") as ps5t:
                h2f = [h2fp.tile([P, CH], F32, tag="h2f", name="h2f")
                       for _ in range(EMB // P)]
                for m in range(NQ):
                    ss2 = r5s.tile([P, 1], F32, tag="ss2", name="ss2")
                    sq5 = r5b.tile([P, EMB], F32, tag="sq5", name="sq5")
                    nc.scalar.activation(
                        sq5[:], xattn[m][:], AF.Square, accum_out=ss2[:]
                    )
                    rt2 = r5s.tile([P, 1], F32, tag="rt2", name="rt2")
                    nc.scalar.activation(
                        rt2[:], ss2[:], AF.Sqrt, bias=eps_t[:], scale=1.0 / EMB
                    )
                    sc2 = r5s.tile([P, 1], F32, tag="sc2", name="sc2")
                    nc.vector.reciprocal(sc2[:], rt2[:])
                    # f32 h2^T via PE transpose (router path)
                    for j in range(EMB // P):
                        xb2 = r5b.tile([P, P], F32, tag="xb2", name="xb2")
                        nc.vector.tensor_scalar(
                            xb2[:], xattn[m][:, j * P : (j + 1) * P], sc2[:],
                            None, op0=ALU.mult,
                        )
                        tp5 = ps5t.tile([P, P], F32, tag="tp5", name="tp5")
                        nc.tensor.transpose(tp5[:], xb2[:], ident_f[:])
                        nc.vector.tensor_copy(h2f[j][:, m * P : (m + 1) * P], tp5[:])
                    # bf16 h2^T via DMA transpose (MoE path)
                    h2b = r5b.tile([P, EMB], BF16, tag="h2b", name="h2b")
                    nc.vector.tensor_scalar(
                        h2b[:], xattn[m][:], sc2[:], None, op0=ALU.mult
                    )
                    nc.sync.dma_start_transpose(
                        h2bf[:, :, m * P : (m + 1) * P], h2b[:]
                    )

                rw_sb = [rwp.tile([P, 8], F32, tag="rw", name="rw") for _ in range(8)]
                for k in range(8):
                    nc.sync.dma_start(rw_sb[k][:], rwT[k])
                combT = combp.tile([NE, CH], F32, tag="combT", name="combT")
                for m in range(NQ):
                    psr = ps5.tile([P, 8], F32, tag="psr", name="psr")
                    for k in range(8):
                        nc.tensor.matmul(
                            psr[:], h2f[k][:, m * P : (m + 1) * P], rw_sb[k][:],
                            start=(k == 0), stop=(k == 7),
                        )
                    negmax = r5s.tile([P, 1], F32, tag="negmax", name="negmax")
                    nc.vector.tensor_reduce(
                        negmax[:], psr[:], axis=AX.X, op=ALU.max, negate=True
                    )
                    et = r5s.tile([P, 8], F32, tag="et", name="et")
                    esum = r5s.tile([P, 1], F32, tag="esum", name="esum")
                    nc.scalar.activation(
                        et[:], psr[:], AF.Exp, bias=negmax[:], accum_out=esum[:]
                    )
                    erec = r5s.tile([P, 1], F32, tag="erec", name="erec")
                    nc.vector.reciprocal(erec[:], esum[:])
                    probs = r5s.tile([P, 8], F32, tag="probs", name="probs")
                    nc.vector.tensor_scalar(probs[:], et[:], erec[:], None, op0=ALU.mult)
                    m1 = r5s.tile([P, 1], F32, tag="m1", name="m1")
                    nc.vector.tensor_reduce(m1[:], probs[:], axis=AX.X, op=ALU.max)
                    ge1 = r5s.tile([P, 8], F32, tag="ge1", name="ge1")
                    nc.vector.tensor_scalar(ge1[:], probs[:], m1[:], None, op0=ALU.is_ge)
                    pm = r5s.tile([P, 8], F32, tag="pm", name="pm")
                    nc.vector.scalar_tensor_tensor(
                        pm[:], ge1[:], -1e9, probs[:], op0=ALU.mult, op1=ALU.add
                    )
                    m2 = r5s.tile([P, 1], F32, tag="m2", name="m2")
                    nc.vector.tensor_reduce(m2[:], pm[:], axis=AX.X, op=ALU.max)
                    den = r5s.tile([P, 1], F32, tag="den", name="den")
                    nc.vector.tensor_tensor(den[:], m1[:], m2[:], op=ALU.add)
                    dr = r5s.tile([P, 1], F32, tag="dr", name="dr")
                    nc.vector.reciprocal(dr[:], den[:])
                    ge2 = r5s.tile([P, 8], F32, tag="ge2", name="ge2")
                    nc.vector.tensor_scalar(ge2[:], probs[:], m2[:], None, op0=ALU.is_ge)
                    comb = r5s.tile([P, 8], F32, tag="comb", name="comb")
                    nc.vector.tensor_scalar(comb[:], probs[:], dr[:], None, op0=ALU.mult)
                    nc.vector.tensor_tensor(comb[:], comb[:], ge2[:], op=ALU.mult)
                    tpc = ps5t.tile([P, P], F32, tag="tp5", name="tpc")
                    nc.tensor.transpose(tpc[:8, :], comb[:], ident_f[:])
                    nc.vector.tensor_copy(combT[:, m * P : (m + 1) * P], tpc[:8, :])
                nc.sync.dma_start(combT_d[:], combT[:])
                for e in range(NE):
                    nc.sync.dma_start(
                        crep[e][:], combT_d[e : e + 1, :].partition_broadcast(P)
                    )

            # ---------- phase 6: MoE mm1 + silu*up*comb -> A (h-major)
            A_p = top.enter_context(tc.tile_pool(name="A", bufs=64))
            A = [A_p.tile([P, CH], BF16, tag="A", name="A") for _ in range(64)]
            with tc.tile_pool(name="w1p", bufs=4) as w1p, \
                 tc.tile_pool(name="sil", bufs=3) as silp, \
                 tc.tile_pool(name="tmp6", bufs=3) as tmp6, \
                 tc.tile_pool(name="ps6", bufs=4, space="PSUM") as ps6:
                for e in range(NE):
                    for j in range(8):
                        w1g = w1p.tile([P, 1024], BF16, tag="w1g", name="w1g")
                        nc.sync.dma_start(w1g[:], w1[e * 16 + j])
                        w1u = w1p.tile([P, 1024], BF16, tag="w1u", name="w1u")
                        nc.sync.dma_start(w1u[:], w1[e * 16 + 8 + j])
                        psg = ps6.tile([P, 512], F32, tag="ps6", name="psg")
                        psu = ps6.tile([P, 512], F32, tag="ps6", name="psu")
                        for k in range(8):
                            nc.tensor.matmul(
                                psg[:], w1g[:, k * P : (k + 1) * P], h2bf[:, k, :],
                                start=(k == 0), stop=(k == 7),
                            )
                        for k in range(8):
                            nc.tensor.matmul(
                                psu[:], w1u[:, k * P : (k + 1) * P], h2bf[:, k, :],
                                start=(k == 0), stop=(k == 7),
                            )
                        sil = silp.tile([P, 512], F32, tag="sil", name="sil")
                        nc.scalar.activation(sil[:], psg[:], AF.Silu)
                        t6 = tmp6.tile([P, 512], F32, tag="t6", name="t6")
                        nc.vector.tensor_tensor(t6[:], sil[:], psu[:], op=ALU.mult)
                        nc.vector.tensor_tensor(
                            A[e * 8 + j][:], t6[:], crep[e][:], op=ALU.mult
                        )

            # ---------- phase 7: MoE mm2 + residual -> y
            with tc.tile_pool(name="w2p", bufs=2) as w2p, \
                 tc.tile_pool(name="yt", bufs=3) as ytp, \
                 tc.tile_pool(name="ps7", bufs=8, space="PSUM") as ps7:
                for n in range(2):
                    pms = [ps7.tile([P, 512], F32, tag="pm7", name="pm7")
                           for _ in range(NQ)]
                    for kg in range(8):
                        w2g = w2p.tile([P, 4096], BF16, tag="w2g", name="w2g")
                        nc.sync.dma_start(w2g[:], w2[n * 8 + kg])
                        for kk in range(8):
                            kt = kg * 8 + kk
                            for m in range(NQ):
                                nc.tensor.matmul(
                                    pms[m][:],
                                    A[kt][:, m * P : (m + 1) * P],
                                    w2g[:, kk * 512 : (kk + 1) * 512],
                                    start=(kt == 0), stop=(kt == 63),
                                )
                    for m in range(NQ):
                        yt = ytp.tile([P, 512], F32, tag="yt", name="yt")
                        nc.vector.tensor_tensor(
                            yt[:], pms[m][:], xattn[m][:, n * 512 : (n + 1) * 512],
                            op=ALU.add,
                        )
                        nc.sync.dma_start(
                            y_out[m * P : (m + 1) * P, n * 512 : (n + 1) * 512], yt[:]
                        )
    return nc


_CACHE: dict = {}


def _get_program(mask_mode: str) -> bass.Bass:
    if mask_mode not in _CACHE:
        _CACHE[mask_mode] = _build(mask_mode)
    return _CACHE[mask_mode]


# ------------------------------------------------------------- host prep
def _prep_weights(norm1_w, norm2_w, q_w, k_w, v_w, o_w, router_w, gate_up, down):
    qwTf = (q_w * norm1_w[None, :]).T.astype(NPBF)  # [EMB, 2048]
    qwT = np.ascontiguousarray(
        qwTf.reshape(8, P, 4, 512).transpose(0, 2, 1, 3)
    )  # [8,4,P,512]
    kwT = np.ascontiguousarray(
        (k_w * norm1_w[None, :]).T.astype(NPBF).reshape(8, P, 512)
    )
    vwT = np.ascontiguousarray(
        (v_w * norm1_w[None, :]).T.astype(NPBF).reshape(8, P, 512)
    )
    owT = np.ascontiguousarray(
        o_w.T.astype(NPBF).reshape(16, P, 2, 512).transpose(0, 2, 1, 3)
    )  # [16,2,P,512]
    rwT = np.ascontiguousarray(
        (router_w * norm2_w[None, :]).T.astype(np.float32)
    ).reshape(8, P, 8)

    w1cat = (gate_up * norm2_w[None, None, :]).reshape(NE * 2 * MH, EMB)
    w1T = w1cat.T.astype(NPBF)  # [EMB, 16384]
    # w1[m][r, k*128+c] = w1T[k*128+r, m*128+c]
    w1 = np.ascontiguousarray(
        w1T.reshape(8, P, 128, P).transpose(2, 1, 0, 3).reshape(128, P, 1024)
    )
    w2cat = down.transpose(0, 2, 1).reshape(NE * MH, EMB).astype(NPBF)  # [8192, EMB]
    # w2[g=n*8+kg][r, kk*512+c] = w2cat[(kg*8+kk)*128+r, n*512+c]
    w2 = np.ascontiguousarray(
        w2cat.reshape(8, 8, P, 2, 512).transpose(3, 0, 2, 1, 4).reshape(16, P, 4096)
    )
    return dict(qwT=qwT, kwT=kwT, vwT=vwT, owT=owT, rwT=rwT, w1=w1, w2=w2)


def _rope_tables(position_ids, qn_w, kn_w):
    pos = np.asarray(position_ids, np.float64).astype(np.float32)  # [S]
    inv = (1.0 / ROPE_BASE ** (np.arange(0, HD, 2, np.float32) / HD)).astype(np.float32)
    fr = pos[:, None] * inv[None, :]  # [S, 64]
    emb = np.concatenate([fr, fr], axis=1)  # [S, HD]
    cos, sin = np.cos(emb), np.sin(emb)
    sign = np.where(np.arange(HD) < HD // 2, -1.0, 1.0).astype(np.float32)
    part = lambda w: np.roll(w, -(HD // 2))  # w[(d+64)%128]
    scl = 1.0 / np.sqrt(HD)
    cosq = (cos * qn_w[None, :] * scl).astype(np.float32)
    sinq = (sin * sign[None, :] * part(qn_w)[None, :] * scl).astype(np.float32)
    cosk = (cos * kn_w[None, :]).astype(np.float32)
    sink = (sin * sign[None, :] * part(kn_w)[None, :]).astype(np.float32)
    return cosq, sinq, cosk, sink


def _prepare(x, position_ids, attn_mask, norm1_w, norm2_w, qn_w, kn_w,
             q_w, k_w, v_w, o_w, router_w, gate_up, down):
    x = np.asarray(x, np.float32)
    mask_full = np.asarray(attn_mask, np.float32)[0, 0]  # [S, S]
    arrs = [np.asarray(a, np.float32) for a in
            (norm1_w, norm2_w, q_w, k_w, v_w, o_w, router_w, gate_up, down)]
    wts = _prep_weights(*arrs)
    cosq, sinq, cosk, sink = _rope_tables(
        position_ids, np.asarray(qn_w, np.float32), np.asarray(kn_w, np.float32)
    )

    mask_mode = "zero" if not mask_full.any() else "general"
    nc = _get_program(mask_mode)

    in_maps = []
    for c in range(8):
        b, i = c // 4, c % 4
        qoff = i * CH
        m = {
            "x": np.ascontiguousarray(np.roll(x[b], -qoff, axis=0)),
            "cosq": np.ascontiguousarray(np.roll(cosq, -qoff, axis=0)[:CH]),
            "sinq": np.ascontiguousarray(np.roll(sinq, -qoff, axis=0)[:CH]),
            "cosk": np.ascontiguousarray(np.roll(cosk, -qoff, axis=0)),
            "sink": np.ascontiguousarray(np.roll(sink, -qoff, axis=0)),
            **wts,
        }
        if mask_mode == "general":
            mrows = mask_full[qoff : qoff + CH, :]
            m["mask"] = np.ascontiguousarray(
                np.roll(mrows, -qoff, axis=1).T.astype(NPBF)
            )
        in_maps.append(m)
    return mask_mode, in_maps


def _assemble(results):
    out = np.empty((B, S, EMB), np.float32)
    for c in range(8):
        b, i = c // 4, c % 4
        out[b, i * CH : (i + 1) * CH, :] = results[c]["y"]
    return out


def kernel(**inputs):
    mask_mode, in_maps = _prepare(**inputs)
    nc = _get_program(mask_mode)
    res = run_bass_kernel_spmd(nc, in_maps, core_ids=list(range(8)))
    return _assemble(res.results)


# revision 16
# speedup vs baseline: 1.0418x; 1.0418x over previous
"""MoE transformer block (attention + top-2 MoE FFN) on 8 Trainium2 cores.

Sharding: token-parallel. Core c handles batch c//4, query chunk (c%4)*512.
Each core receives its batch's tokens ROLLED so that its query chunk sits at
rows 0..511 — the compiled program is identical across cores (pure SPMD) and
all per-core variation lives in the input data (x, rope tables, mask columns).

Host-side folding: norm1_w into q/k/v weights, norm2_w into router/gate_up,
q/k-norm weights and the 1/sqrt(HD) score scale into the rope cos/sin tables.
Matmuls run in bf16 with f32 PSUM accumulation; softmax and rmsnorm run in
f32; the router path (h2 -> logits) stays f32 so top-2 expert selection
matches the f32 reference.  MoE is computed densely (all 8 experts) as two
stacked matmuls; the top-2 combine weights are zero for unselected experts
and are folded into the activation in expert-major layout.  All bf16
activation transposes go through the DMA xbar (dma_start_transpose), keeping
PE/DVE free for matmuls and evictions.
"""

import sys
from contextlib import ExitStack

sys.path.insert(0, "/opt/trn_rl_repo")

import numpy as np
import ml_dtypes

import concourse.bass as bass
import concourse.mybir as mybir
import concourse.tile as tile
from concourse.vector_clock import ScopedClock
from concourse.masks import make_identity
from concourse.bass_utils import run_bass_kernel_spmd

# ---------------------------------------------------------------- constants
B, S, EMB = 2, 2048, 1024
NH, NKV, HD = 16, 4, 128
NE, MH = 8, 1024
CH = 512  # query tokens per core
P = 128
NT = S // P  # 16 token tiles
NQ = CH // P  # 4 query tiles
EPS = 1e-6
ROPE_BASE = 10000.0

F32 = mybir.dt.float32
BF16 = mybir.dt.bfloat16
AF = mybir.ActivationFunctionType
ALU = mybir.AluOpType
AX = mybir.AxisListType
NPBF = ml_dtypes.bfloat16

# ------------------------------------------------- walrus single-wait patch
_uid = [0]


class _SplitWaitTileContext(tile.TileContext):
    """This container's walrus build rejects instructions carrying more than
    one sync wait; hoist extra waits onto same-engine single-wait NoOps."""

    def _add_instruction(self, inst):
        si = inst.sync_info
        if si is not None and len(si.on_wait) > 1:
            waits = list(si.on_wait)
            for w in waits[:-1]:
                _uid[0] += 1
                nop = mybir.InstNoOp(
                    name=f"WSPLIT-{_uid[0]}",
                    engine=inst.engine,
                    ins=[],
                    outs=[],
                    sync_info=mybir.SyncInfo(on_wait=[w], on_update=[]),
                )
                super()._add_instruction(nop)
            inst.sync_info = mybir.SyncInfo(
                on_wait=[waits[-1]], on_update=list(si.on_update)
            )
        super()._add_instruction(inst)

    def _drain_and_barrier(self, tick_clock, wait_clock):
        nc = self.nc
        drain_inst = nc.sync.drain()
        wait_clock.add_sem_waits(
            drain_inst.ins, ScopedClock({None: tick_clock.global_clock})
        )
        si = drain_inst.ins.sync_info
        if si is not None and len(si.on_wait) > 1:
            waits = list(si.on_wait)
            drain_inst.ins.sync_info = mybir.SyncInfo(
                on_wait=[waits[0]], on_update=list(si.on_update)
            )
            for w in waits[1:]:
                nop = nc.sync.nop(nofuse=True)
                nop.ins.sync_info = mybir.SyncInfo(on_wait=[w], on_update=[])
        nc.all_engine_barrier()
        assert self.sems is not None
        popped = nc._tile_sem_poison_stack.pop()
        assert popped is self._sem_poison
        nc.clear_and_free_semaphores(list(self.sems.allocated().values()))
        nc.all_engine_barrier()


# ------------------------------------------------------------ program build
def _build(mask_mode: str) -> bass.Bass:
    """mask_mode: 'zero' (mask known all-zero, skip the add) or 'general'."""
    nc = bass.Bass()

    x_in = nc.declare_dram_parameter("x", [S, EMB], F32, isOutput=False)
    cosq = nc.declare_dram_parameter("cosq", [CH, HD], F32, isOutput=False)
    sinq = nc.declare_dram_parameter("sinq", [CH, HD], F32, isOutput=False)
    cosk = nc.declare_dram_parameter("cosk", [S, HD], F32, isOutput=False)
    sink = nc.declare_dram_parameter("sink", [S, HD], F32, isOutput=False)
    qwT = nc.declare_dram_parameter("qwT", [8, 4, P, 512], BF16, isOutput=False)
    kwT = nc.declare_dram_parameter("kwT", [8, P, 512], BF16, isOutput=False)
    vwT = nc.declare_dram_parameter("vwT", [8, P, 512], BF16, isOutput=False)
    owT = nc.declare_dram_parameter("owT", [16, 2, P, 512], BF16, isOutput=False)
    rwT = nc.declare_dram_parameter("rwT", [8, P, 8], F32, isOutput=False)
    w1 = nc.declare_dram_parameter("w1", [128, P, 1024], BF16, isOutput=False)
    w2 = nc.declare_dram_parameter("w2", [16, P, 4096], BF16, isOutput=False)
    if mask_mode == "general":
        mask_in = nc.declare_dram_parameter("mask", [S, CH], BF16, isOutput=False)
    y_out = nc.declare_dram_parameter("y", [CH, EMB], F32, isOutput=True)



    with _SplitWaitTileContext(nc) as tc:
        with ExitStack() as top:
            const = top.enter_context(tc.tile_pool(name="const", bufs=1))
            ident_f = const.tile([P, P], F32, tag="identf", name="identf")
            make_identity(nc, ident_f)
            eps_t = const.tile([P, 1], F32, tag="epst", name="epst")
            nc.vector.memset(eps_t[:], EPS)
            ones_bf = const.tile([P, 1], BF16, tag="onesbf", name="onesbf")
            nc.vector.memset(ones_bf[:], 1.0)
            dram_p = top.enter_context(
                tc.tile_pool(name="dram", bufs=1, space="DRAM"))
            combT_d = dram_p.tile([NE, CH], F32, tag="combTd", name="combTd")
            rcp_d = dram_p.tile([NH, CH], F32, tag="rcpd", name="rcpd")

            # persistent across attention
            xattn_p = top.enter_context(tc.tile_pool(name="xattn", bufs=NQ))
            xattn = [xattn_p.tile([P, EMB], F32, tag="xattn", name="xattn")
                     for _ in range(NQ)]

            with ExitStack() as attn_stack:
                ctxT_p = attn_stack.enter_context(tc.tile_pool(name="ctxT", bufs=NH))
                ctxT = [ctxT_p.tile([P, CH], BF16, tag="ctxT", name="ctxT")
                        for _ in range(NH)]

                with ExitStack() as qkv_stack:
                    kvq_p = qkv_stack.enter_context(tc.tile_pool(name="kvq", bufs=1))
                    kT = kvq_p.tile([P, NKV, S], BF16, tag="kTb", name="kTb")
                    vB = kvq_p.tile([P, NT, 512], BF16, tag="vB", name="vB")
                    qT = kvq_p.tile([P, NH, CH], BF16, tag="qTb", name="qTb")

                    # ---------- phase 1: rmsnorm(x) -> xhatT (bf16 feature-major)
                    with ExitStack() as ph1:
                        xh_p = ph1.enter_context(tc.tile_pool(name="xhT", bufs=1))
                        xhatT = xh_p.tile([P, EMB // P, S], BF16, tag="xhT", name="xhT")
                        with tc.tile_pool(name="ph1s", bufs=3) as sp, \
                             tc.tile_pool(name="ph1b", bufs=3) as bp, \
                             tc.tile_pool(name="ph1ss", bufs=4) as ssp:
                            for t in range(NT):
                                xt = sp.tile([P, EMB], F32, tag="xt", name="xt")
                                nc.sync.dma_start(xt[:], x_in[t * P : (t + 1) * P, :])
                                ss = ssp.tile([P, 1], F32, tag="ss", name="ss")
                                sq1 = sp.tile([P, EMB], F32, tag="sq1", name="sq1")
                                nc.scalar.activation(
                                    sq1[:], xt[:], AF.Square, accum_out=ss[:]
                                )
                                rt = ssp.tile([P, 1], F32, tag="rt", name="rt")
                                nc.scalar.activation(
                                    rt[:], ss[:], AF.Sqrt, bias=eps_t[:], scale=1.0 / EMB
                                )
                                sc = ssp.tile([P, 1], F32, tag="sc", name="sc")
                                nc.vector.reciprocal(sc[:], rt[:])
                                xb = bp.tile([P, EMB], BF16, tag="xb", name="xb")
                                nc.vector.tensor_scalar(
                                    xb[:], xt[:], sc[:], None, op0=ALU.mult
                                )
                                nc.sync.dma_start_transpose(
                                    xhatT[:, :, t * P : (t + 1) * P], xb[:]
                                )

                        # ---------- phase 2: Q/K/V projections (+norm+rope+T)
                        with tc.tile_pool(name="tabs", bufs=NT) as tabp, \
                             tc.tile_pool(name="kwp", bufs=8) as kwp, \
                             tc.tile_pool(name="vwp", bufs=8) as vwp, \
                             tc.tile_pool(name="qwp", bufs=8) as qwp, \
                             tc.tile_pool(name="kvf", bufs=3) as kvf, \
                             tc.tile_pool(name="rope", bufs=6) as rp, \
                             tc.tile_pool(name="ropss", bufs=8) as rssp, \
                             tc.tile_pool(name="hbf", bufs=3) as hbfp, \
                             tc.tile_pool(name="kvps", bufs=4, space="PSUM") as kvps:
                            coskt = [tabp.tile([P, HD], F32, tag="coskt", name="coskt")
                                     for _ in range(NT)]
                            sinkt = [tabp.tile([P, HD], F32, tag="sinkt", name="sinkt")
                                     for _ in range(NT)]
                            cosqt = [tabp.tile([P, HD], F32, tag="cosqt", name="cosqt")
                                     for _ in range(NQ)]
                            sinqt = [tabp.tile([P, HD], F32, tag="sinqt", name="sinqt")
                                     for _ in range(NQ)]
                            for t in range(NT):
                                nc.sync.dma_start(coskt[t][:], cosk[t * P : (t + 1) * P, :])
                                nc.sync.dma_start(sinkt[t][:], sink[t * P : (t + 1) * P, :])
                            for m in range(NQ):
                                nc.sync.dma_start(cosqt[m][:], cosq[m * P : (m + 1) * P, :])
                                nc.sync.dma_start(sinqt[m][:], sinq[m * P : (m + 1) * P, :])

                            kw_sb = [kwp.tile([P, 512], BF16, tag="kw", name="kw")
                                     for _ in range(8)]
                            vw_sb = [vwp.tile([P, 512], BF16, tag="vw", name="vw")
                                     for _ in range(8)]
                            for k in range(8):
                                nc.sync.dma_start(kw_sb[k][:], kwT[k])
                                nc.sync.dma_start(vw_sb[k][:], vwT[k])

                            def norm_rope(src, cost, sint, dst):
                                """src [P,HD] f32 -> rmsnorm+rope -> bf16 into dst."""
                                ssq = rssp.tile([P, 1], F32, tag="ssq", name="ssq")
                                sqr = rp.tile([P, HD], F32, tag="sqr", name="sqr")
                                nc.scalar.activation(
                                    sqr[:], src, AF.Square, accum_out=ssq[:]
                                )
                                rtq = rssp.tile([P, 1], F32, tag="rtq", name="rtq")
                                nc.scalar.activation(
                                    rtq[:], ssq[:], AF.Sqrt, bias=eps_t[:], scale=1.0 / HD
                                )
                                scq = rssp.tile([P, 1], F32, tag="scq", name="scq")
                                nc.vector.reciprocal(scq[:], rtq[:])
                                tcos = rp.tile([P, HD], F32, tag="tcos", name="tcos")
                                nc.vector.tensor_tensor(tcos[:], src, cost[:], op=ALU.mult)
                                tsin = rp.tile([P, HD], F32, tag="tsin", name="tsin")
                                h = HD // 2
                                nc.vector.tensor_tensor(
                                    tsin[:, :h], src[:, h:], sint[:, :h], op=ALU.mult
                                )
                                nc.vector.tensor_tensor(
                                    tsin[:, h:], src[:, :h], sint[:, h:], op=ALU.mult
                                )
                                t1 = rp.tile([P, HD], F32, tag="t1", name="t1")
                                nc.vector.tensor_scalar(
                                    t1[:], tcos[:], scq[:], None, op0=ALU.mult
                                )
                                nc.vector.scalar_tensor_tensor(
                                    dst, tsin[:], scq[:], t1[:],
                                    op0=ALU.mult, op1=ALU.add,
                                )

                            # K and V over all token tiles
                            for t in range(NT):
                                ps_k = kvps.tile([P, 512], F32, tag="ps2", name="psk")
                                ps_v = kvps.tile([P, 512], F32, tag="ps2", name="psv")
                                for k in range(8):
                                    nc.tensor.matmul(
                                        ps_k[:],
                                        xhatT[:, k, t * P : (t + 1) * P],
                                        kw_sb[k][:],
                                        start=(k == 0), stop=(k == 7),
                                    )
                                for k in range(8):
                                    nc.tensor.matmul(
                                        ps_v[:],
                                        xhatT[:, k, t * P : (t + 1) * P],
                                        vw_sb[k][:],
                                        start=(k == 0), stop=(k == 7),
                                    )
                                kf = kvf.tile([P, 512], F32, tag="kf", name="kf")
                                nc.vector.tensor_copy(kf[:], ps_k[:])
                                khat = hbfp.tile([P, 512], BF16, tag="khat", name="khat")
                                for kv in range(NKV):
                                    norm_rope(
                                        kf[:, kv * HD : (kv + 1) * HD],
                                        coskt[t], sinkt[t],
                                        khat[:, kv * HD : (kv + 1) * HD],
                                    )
                                nc.sync.dma_start_transpose(
                                    kT[:, :, t * P : (t + 1) * P], khat[:]
                                )
                                nc.vector.tensor_copy(vB[:, t, :], ps_v[:])

                            # Q over the query chunk
                            for hg in range(4):
                                qw_sb = [qwp.tile([P, 512], BF16, tag="qw", name="qw")
                                         for _ in range(8)]
                                for k in range(8):
                                    nc.sync.dma_start(qw_sb[k][:], qwT[k, hg])
                                for m in range(NQ):
                                    ps_q = kvps.tile([P, 512], F32, tag="ps2", name="psq")
                                    for k in range(8):
                                        nc.tensor.matmul(
                                            ps_q[:],
                                            xhatT[:, k, m * P : (m + 1) * P],
                                            qw_sb[k][:],
                                            start=(k == 0), stop=(k == 7),
                                        )
                                    qf = kvf.tile([P, 512], F32, tag="qf", name="qf")
                                    nc.vector.tensor_copy(qf[:], ps_q[:])
                                    qhat = hbfp.tile([P, 512], BF16, tag="qhat", name="qhat")
                                    for hh in range(4):
                                        norm_rope(
                                            qf[:, hh * HD : (hh + 1) * HD],
                                            cosqt[m], sinqt[m],
                                            qhat[:, hh * HD : (hh + 1) * HD],
                                        )
                                    nc.sync.dma_start_transpose(
                                        qT[:, hg * 4 : (hg + 1) * 4, m * P : (m + 1) * P],
                                        qhat[:],
                                    )
                    # xhatT freed here

                    # ---------- phase 3: attention per head (k-major scores,
                    # exp gives attn^T directly; rowsums via ones-matmul)
                    with ExitStack() as ph3:
                        if mask_mode == "general":
                            mk_p = ph3.enter_context(tc.tile_pool(name="mask", bufs=NT))
                            mkT = [mk_p.tile([P, CH], BF16, tag="mkT", name="mkT")
                                   for _ in range(NT)]
                            for kt in range(NT):
                                nc.sync.dma_start(
                                    mkT[kt][:], mask_in[kt * P : (kt + 1) * P, :]
                                )
                        attnT_p = ph3.enter_context(tc.tile_pool(name="attnT", bufs=2))
                        sc_p = ph3.enter_context(tc.tile_pool(name="scf", bufs=4))
                        rr_p = ph3.enter_context(tc.tile_pool(name="rr", bufs=4))
                        rep_p = ph3.enter_context(tc.tile_pool(name="rep", bufs=2))
                        ps_s = ph3.enter_context(
                            tc.tile_pool(name="pss", bufs=3, space="PSUM"))
                        ps_c = ph3.enter_context(
                            tc.tile_pool(name="psc", bufs=2, space="PSUM"))
                        ps_r = ph3.enter_context(
                            tc.tile_pool(name="psr3", bufs=2, space="PSUM"))

                        for h in range(NH):
                            kv = h // (NH // NKV)
                            attnT = attnT_p.tile([P, NT, CH], BF16, tag="attnT",
                                                 name="attnT")
                            ps_sum = ps_r.tile([1, CH], F32, tag="psum3", name="psum3")
                            for kt in range(NT):
                                pss = ps_s.tile([P, CH], F32, tag="pss", name="pss")
                                nc.tensor.matmul(
                                    pss[:],
                                    kT[:, kv, kt * P : (kt + 1) * P],
                                    qT[:, h, :],
                                    start=True, stop=True,
                                )
                                if mask_mode == "general":
                                    scf = sc_p.tile([P, CH], F32, tag="scf", name="scf")
                                    nc.vector.tensor_tensor(
                                        scf[:], pss[:], mkT[kt][:], op=ALU.add
                                    )
                                    src3 = scf
                                else:
                                    src3 = pss
                                nc.scalar.activation(
                                    attnT[:, kt, :], src3[:], AF.Exp
                                )
                                nc.tensor.matmul(
                                    ps_sum[:], ones_bf[:], attnT[:, kt, :],
                                    start=(kt == 0), stop=(kt == NT - 1),
                                )
                            rcp_row = rr_p.tile([1, CH], F32, tag="rcpr", name="rcpr")
                            nc.vector.reciprocal(rcp_row[:], ps_sum[:])
                            nc.sync.dma_start(rcp_d[h : h + 1, :], rcp_row[:])
                            rcp_rep = rep_p.tile([P, CH], F32, tag="rcprep",
                                                 name="rcprep")
                            nc.sync.dma_start(
                                rcp_rep[:], rcp_d[h : h + 1, :].partition_broadcast(P)
                            )
                            psc = ps_c.tile([P, CH], F32, tag="psc", name="psc")
                            for kt in range(NT):
                                nc.tensor.matmul(
                                    psc[:],
                                    vB[:, kt, kv * P : (kv + 1) * P],
                                    attnT[:, kt, :],
                                    start=(kt == 0), stop=(kt == NT - 1),
                                )
                            nc.vector.tensor_tensor(
                                ctxT[h][:], psc[:], rcp_rep[:], op=ALU.mult
                            )
                # kT / vB / qT freed here

                # ---------- phase 4: o_proj + residual
                with tc.tile_pool(name="ow", bufs=16) as owp, \
                     tc.tile_pool(name="xq", bufs=NQ) as xqp, \
                     tc.tile_pool(name="pso", bufs=3, space="PSUM") as pso:
                    xq = [xqp.tile([P, EMB], F32, tag="xq", name="xq")
                          for _ in range(NQ)]
                    for m in range(NQ):
                        nc.sync.dma_start(xq[m][:], x_in[m * P : (m + 1) * P, :])
                    for n in range(2):
                        ow_sb = [owp.tile([P, 512], BF16, tag="ow", name="ow")
                                 for _ in range(16)]
                        for k in range(16):
                            nc.sync.dma_start(ow_sb[k][:], owT[k, n])
                        for m in range(NQ):
                            ps = pso.tile([P, 512], F32, tag="pso", name="pso")
                            for k in range(16):
                                nc.tensor.matmul(
                                    ps[:],
                                    ctxT[k][:, m * P : (m + 1) * P],
                                    ow_sb[k][:],
                                    start=(k == 0), stop=(k == 15),
                                )
                            nc.vector.tensor_tensor(
                                xattn[m][:, n * 512 : (n + 1) * 512],
                                ps[:], xq[m][:, n * 512 : (n + 1) * 512],
                                op=ALU.add,
                            )
            # ctxT freed here

            # ---------- phase 5: h2, router, top-2 comb
            h2bf_p = top.enter_context(tc.tile_pool(name="h2bf", bufs=1))
            h2bf = h2bf_p.tile([P, EMB // P, CH], BF16, tag="h2bf", name="h2bf")
            crep_p = top.enter_context(tc.tile_pool(name="crep", bufs=NE))
            crep = [crep_p.tile([P, CH], F32, tag="crep", name="crep")
                    for _ in range(NE)]

            with tc.tile_pool(name="h2f", bufs=EMB // P) as h2fp, \
                 tc.tile_pool(name="rw", bufs=8) as rwp, \
                 tc.tile_pool(name="r5s", bufs=8) as r5s, \
                 tc.tile_pool(name="r5b", bufs=3) as r5b, \
                 tc.tile_pool(name="combT", bufs=1) as combp, \
                 tc.tile_pool(name="ps5", bufs=2, space="PSUM") as ps5, \
                 tc.tile_pool(name="ps5t", bufs=2, space="PSUM") as ps5t:
                h2f = [h2fp.tile([P, CH], F32, tag="h2f", name="h2f")
                       for _ in range(EMB // P)]
                for m in range(NQ):
                    ss2 = r5s.tile([P, 1], F32, tag="ss2", name="ss2")
                    sq5 = r5b.tile([P, EMB], F32, tag="sq5", name="sq5")
                    nc.scalar.activation(
                        sq5[:], xattn[m][:], AF.Square, accum_out=ss2[:]
                    )
                    rt2 = r5s.tile([P, 1], F32, tag="rt2", name="rt2")
                    nc.scalar.activation(
                        rt2[:], ss2[:], AF.Sqrt, bias=eps_t[:], scale=1.0 / EMB
                    )
                    sc2 = r5s.tile([P, 1], F32, tag="sc2", name="sc2")
                    nc.vector.reciprocal(sc2[:], rt2[:])
                    # f32 h2^T via PE transpose (router path)
                    for j in range(EMB // P):
                        xb2 = r5b.tile([P, P], F32, tag="xb2", name="xb2")
                        nc.vector.tensor_scalar(
                            xb2[:], xattn[m][:, j * P : (j + 1) * P], sc2[:],
                            None, op0=ALU.mult,
                        )
                        tp5 = ps5t.tile([P, P], F32, tag="tp5", name="tp5")
                        nc.tensor.transpose(tp5[:], xb2[:], ident_f[:])
                        nc.vector.tensor_copy(h2f[j][:, m * P : (m + 1) * P], tp5[:])
                    # bf16 h2^T via DMA transpose (MoE path)
                    h2b = r5b.tile([P, EMB], BF16, tag="h2b", name="h2b")
                    nc.vector.tensor_scalar(
                        h2b[:], xattn[m][:], sc2[:], None, op0=ALU.mult
                    )
                    nc.sync.dma_start_transpose(
                        h2bf[:, :, m * P : (m + 1) * P], h2b[:]
                    )

                rw_sb = [rwp.tile([P, 8], F32, tag="rw", name="rw") for _ in range(8)]
                for k in range(8):
                    nc.sync.dma_start(rw_sb[k][:], rwT[k])
                combT = combp.tile([NE, CH], F32, tag="combT", name="combT")
                for m in range(NQ):
                    psr = ps5.tile([P, 8], F32, tag="psr", name="psr")
                    for k in range(8):
                        nc.tensor.matmul(
                            psr[:], h2f[k][:, m * P : (m + 1) * P], rw_sb[k][:],
                            start=(k == 0), stop=(k == 7),
                        )
                    negmax = r5s.tile([P, 1], F32, tag="negmax", name="negmax")
                    nc.vector.tensor_reduce(
                        negmax[:], psr[:], axis=AX.X, op=ALU.max, negate=True
                    )
                    et = r5s.tile([P, 8], F32, tag="et", name="et")
                    esum = r5s.tile([P, 1], F32, tag="esum", name="esum")
                    nc.scalar.activation(
                        et[:], psr[:], AF.Exp, bias=negmax[:], accum_out=esum[:]
                    )
                    erec = r5s.tile([P, 1], F32, tag="erec", name="erec")
                    nc.vector.reciprocal(erec[:], esum[:])
                    probs = r5s.tile([P, 8], F32, tag="probs", name="probs")
                    nc.vector.tensor_scalar(probs[:], et[:], erec[:], None, op0=ALU.mult)
                    m1 = r5s.tile([P, 1], F32, tag="m1", name="m1")
                    nc.vector.tensor_reduce(m1[:], probs[:], axis=AX.X, op=ALU.max)
                    ge1 = r5s.tile([P, 8], F32, tag="ge1", name="ge1")
                    nc.vector.tensor_scalar(ge1[:], probs[:], m1[:], None, op0=ALU.is_ge)
                    pm = r5s.tile([P, 8], F32, tag="pm", name="pm")
                    nc.vector.scalar_tensor_tensor(
                        pm[:], ge1[:], -1e9, probs[:], op0=ALU.mult, op1=ALU.add
                    )
                    m2 = r5s.tile([P, 1], F32, tag="m2", name="m2")
                    nc.vector.tensor_reduce(m2[:], pm[:], axis=AX.X, op=ALU.max)
                    den = r5s.tile([P, 1], F32, tag="den", name="den")
                    nc.vector.tensor_tensor(den[:], m1[:], m2[:], op=ALU.add)
                    dr = r5s.tile([P, 1], F32, tag="dr", name="dr")
                    nc.vector.reciprocal(dr[:], den[:])
                    ge2 = r5s.tile([P, 8], F32, tag="ge2", name="ge2")
                    nc.vector.tensor_scalar(ge2[:], probs[:], m2[:], None, op0=ALU.is_ge)
                    comb = r5s.tile([P, 8], F32, tag="comb", name="comb")
                    nc.vector.tensor_scalar(comb[:], probs[:], dr[:], None, op0=ALU.mult)
                    nc.vector.tensor_tensor(comb[:], comb[:], ge2[:], op=ALU.mult)
                    tpc = ps5t.tile([P, P], F32, tag="tp5", name="tpc")
                    nc.tensor.transpose(tpc[:8, :], comb[:], ident_f[:])
                    nc.vector.tensor_copy(combT[:, m * P : (m + 1) * P], tpc[:8, :])
                nc.sync.dma_start(combT_d[:], combT[:])
                for e in range(NE):
                    nc.sync.dma_start(
                        crep[e][:], combT_d[e : e + 1, :].partition_broadcast(P)
                    )

            # ---------- phase 6: MoE mm1 + silu*up*comb -> A (h-major)
            A_p = top.enter_context(tc.tile_pool(name="A", bufs=64))
            A = [A_p.tile([P, CH], BF16, tag="A", name="A") for _ in range(64)]
            with tc.tile_pool(name="w1p", bufs=4) as w1p, \
                 tc.tile_pool(name="sil", bufs=3) as silp, \
                 tc.tile_pool(name="tmp6", bufs=3) as tmp6, \
                 tc.tile_pool(name="ps6", bufs=4, space="PSUM") as ps6:
                for e in range(NE):
                    for j in range(8):
                        w1g = w1p.tile([P, 1024], BF16, tag="w1g", name="w1g")
                        nc.sync.dma_start(w1g[:], w1[e * 16 + j])
                        w1u = w1p.tile([P, 1024], BF16, tag="w1u", name="w1u")
                        nc.sync.dma_start(w1u[:], w1[e * 16 + 8 + j])
                        psg = ps6.tile([P, 512], F32, tag="ps6", name="psg")
                        psu = ps6.tile([P, 512], F32, tag="ps6", name="psu")
                        for k in range(8):
                            nc.tensor.matmul(
                                psg[:], w1g[:, k * P : (k + 1) * P], h2bf[:, k, :],
                                start=(k == 0), stop=(k == 7),
                            )
                        for k in range(8):
                            nc.tensor.matmul(
                                psu[:], w1u[:, k * P : (k + 1) * P], h2bf[:, k, :],
                                start=(k == 0), stop=(k == 7),
                            )
                        sil = silp.tile([P, 512], F32, tag="sil", name="sil")
                        nc.scalar.activation(sil[:], psg[:], AF.Silu)
                        t6 = tmp6.tile([P, 512], F32, tag="t6", name="t6")
                        nc.vector.tensor_tensor(t6[:], sil[:], psu[:], op=ALU.mult)
                        nc.vector.tensor_tensor(
                            A[e * 8 + j][:], t6[:], crep[e][:], op=ALU.mult
                        )

            # ---------- phase 7: MoE mm2 + residual -> y
            with tc.tile_pool(name="w2p", bufs=2) as w2p, \
                 tc.tile_pool(name="yt", bufs=3) as ytp, \
                 tc.tile_pool(name="ps7", bufs=8, space="PSUM") as ps7:
                for n in range(2):
                    pms = [ps7.tile([P, 512], F32, tag="pm7", name="pm7")
                           for _ in range(NQ)]
                    for kg in range(8):
                        w2g = w2p.tile([P, 4096], BF16, tag="w2g", name="w2g")
                        nc.sync.dma_start(w2g[:], w2[n * 8 + kg])
                        for kk in range(8):
                            kt = kg * 8 + kk
                            for m in range(NQ):
                                nc.tensor.matmul(
                                    pms[m][:],
                                    A[kt][:, m * P : (m + 1) * P],
                                    w2g[:, kk * 512 : (kk + 1) * 512],
                                    start=(kt == 0), stop=(kt == 63),
                                )
                    for m in range(NQ):
                        yt = ytp.tile([P, 512], F32, tag="yt", name="yt")
                        nc.vector.tensor_tensor(
                            yt[:], pms[m][:], xattn[m][:, n * 512 : (n + 1) * 512],
                            op=ALU.add,
                        )
                        nc.sync.dma_start(
                            y_out[m * P : (m + 1) * P, n * 512 : (n + 1) * 512], yt[:]
                        )
    return nc


_CACHE: dict = {}


def _get_program(mask_mode: str) -> bass.Bass:
    if mask_mode not in _CACHE:
        _CACHE[mask_mode] = _build(mask_mode)
    return _CACHE[mask_mode]


# ------------------------------------------------------------- host prep
def _prep_weights(norm1_w, norm2_w, q_w, k_w, v_w, o_w, router_w, gate_up, down):
    qwTf = (q_w * norm1_w[None, :]).T.astype(NPBF)  # [EMB, 2048]
    qwT = np.ascontiguousarray(
        qwTf.reshape(8, P, 4, 512).transpose(0, 2, 1, 3)
    )  # [8,4,P,512]
    kwT = np.ascontiguousarray(
        (k_w * norm1_w[None, :]).T.astype(NPBF).reshape(8, P, 512)
    )
    vwT = np.ascontiguousarray(
        (v_w * norm1_w[None, :]).T.astype(NPBF).reshape(8, P, 512)
    )
    owT = np.ascontiguousarray(
        o_w.T.astype(NPBF).reshape(16, P, 2, 512).transpose(0, 2, 1, 3)
    )  # [16,2,P,512]
    rwT = np.ascontiguousarray(
        (router_w * norm2_w[None, :]).T.astype(np.float32)
    ).reshape(8, P, 8)

    w1cat = (gate_up * norm2_w[None, None, :]).reshape(NE * 2 * MH, EMB)
    w1T = w1cat.T.astype(NPBF)  # [EMB, 16384]
    # w1[m][r, k*128+c] = w1T[k*128+r, m*128+c]
    w1 = np.ascontiguousarray(
        w1T.reshape(8, P, 128, P).transpose(2, 1, 0, 3).reshape(128, P, 1024)
    )
    w2cat = down.transpose(0, 2, 1).reshape(NE * MH, EMB).astype(NPBF)  # [8192, EMB]
    # w2[g=n*8+kg][r, kk*512+c] = w2cat[(kg*8+kk)*128+r, n*512+c]
    w2 = np.ascontiguousarray(
        w2cat.reshape(8, 8, P, 2, 512).transpose(3, 0, 2, 1, 4).reshape(16, P, 4096)
    )
    return dict(qwT=qwT, kwT=kwT, vwT=vwT, owT=owT, rwT=rwT, w1=w1, w2=w2)


def _rope_tables(position_ids, qn_w, kn_w):
    pos = np.asarray(position_ids, np.float64).astype(np.float32)  # [S]
    inv = (1.0 / ROPE_BASE ** (np.arange(0, HD, 2, np.float32) / HD)).astype(np.float32)
    fr = pos[:, None] * inv[None, :]  # [S, 64]
    emb = np.concatenate([fr, fr], axis=1)  # [S, HD]
    cos, sin = np.cos(emb), np.sin(emb)
    sign = np.where(np.arange(HD) < HD // 2, -1.0, 1.0).astype(np.float32)
    part = lambda w: np.roll(w, -(HD // 2))  # w[(d+64)%128]
    scl = 1.0 / np.sqrt(HD)
    cosq = (cos * qn_w[None, :] * scl).astype(np.float32)
    sinq = (sin * sign[None, :] * part(qn_w)[None, :] * scl).astype(np.float32)
    cosk = (cos * kn_w[None, :]).astype(np.float32)
    sink = (sin * sign[None, :] * part(kn_w)[None, :]).astype(np.float32)
    return cosq, sinq, cosk, sink


def _prepare(x, position_ids, attn_mask, norm1_w, norm2_w, qn_w, kn_w,
             q_w, k_w, v_w, o_w, router_w, gate_up, down):
    x = np.asarray(x, np.float32)
    mask_full = np.asarray(attn_mask, np.float32)[0, 0]  # [S, S]
    arrs = [np.asarray(a, np.float32) for a in
            (norm1_w, norm2_w, q_w, k_w, v_w, o_w, router_w, gate_up, down)]
    wts = _prep_weights(*arrs)
    cosq, sinq, cosk, sink = _rope_tables(
        position_ids, np.asarray(qn_w, np.float32), np.asarray(kn_w, np.float32)
    )

    mask_mode = "zero" if not mask_full.any() else "general"
    nc = _get_program(mask_mode)

    in_maps = []
    for c in range(8):
        b, i = c // 4, c % 4
        qoff = i * CH
        m = {
            "x": np.ascontiguousarray(np.roll(x[b], -qoff, axis=0)),
            "cosq": np.ascontiguousarray(np.roll(cosq, -qoff, axis=0)[:CH]),
            "sinq": np.ascontiguousarray(np.roll(sinq, -qoff, axis=0)[:CH]),
            "cosk": np.ascontiguousarray(np.roll(cosk, -qoff, axis=0)),
            "sink": np.ascontiguousarray(np.roll(sink, -qoff, axis=0)),
            **wts,
        }
        if mask_mode == "general":
            mrows = mask_full[qoff : qoff + CH, :]
            m["mask"] = np.ascontiguousarray(
                np.roll(mrows, -qoff, axis=1).T.astype(NPBF)
            )
        in_maps.append(m)
    return mask_mode, in_maps


def _assemble(results):
    out = np.empty((B, S, EMB), np.float32)
    for c in range(8):
        b, i = c // 4, c % 4
        out[b, i * CH : (i + 1) * CH, :] = results[c]["y"]
    return out


def kernel(**inputs):
    mask_mode, in_maps = _prepare(**inputs)
    nc = _get_program(mask_mode)
    res = run_bass_kernel_spmd(nc, in_maps, core_ids=list(range(8)))
    return _assemble(res.results)
